# revision 39
# baseline (speedup 1.0000x reference)
"""Causal self-attention (B=4, S=2048, D=1024, single 1024-wide head) on 8 TRN2 cores.

Sharding: core c -> batch b=c//2, parity h=c%2; each core computes the 8
query blocks {h, h+2, ..., h+14} (128 rows each) of its batch. Key-extents
padded to 256*(j+1) keep the program identical on every core; causality
lives in per-core additive-mask input data, not control flow.

The key algebraic move: with zero q/k biases, scores = Q K^T =
x (Wq^T Wk) x^T, so M = Wq^T Wk is folded ON THE HOST (weights-only) and
the device needs no K projection and no Q projection -- just one
XM = x M projection of the core's own queries (ec-outer over 4 PSUM groups,
paced behind the initial DMA), then scores contract XM^T against the raw
x^T. V is projected in full per core. Everything is local: no collectives
(a pair-wise AllGather costs ~45us on the serialized CC ring, which is why
sharded K/V projections lose), no readbacks, ~10MB of input DMA.

Phases: XM projection -> V projection (the two biggest s_phases hoisted
into its tail so their exp/transpose latencies hide under V matmuls) ->
attention, software-pipelined with the tail run 2 s_phases ahead so no
av_phase waits on an exp->transpose chain. Loads are issued in exact
consumption order at half-tile granularity (DMA queues are FIFO).

All matmuls run on the PE in bf16 with fp32 PSUM accumulation. Softmax
skips max-subtraction (scores are ~N(0,1); exp stays in fp32 range) so the
denominator comes free from the Exp activation's accumulate output.
"""

import time

import numpy as np
import ml_dtypes

import concourse.bass as bass
import concourse.bacc as bacc
import concourse.tile as tile
from concourse import mybir
from concourse import bass_utils

BF16 = ml_dtypes.bfloat16
P = 128
B, S, D = 4, 2048, 1024
H = S // 2   # keys owned per core (half a batch)
EC = D // P  # contraction chunks (8)
NQB = 8      # query blocks per core
NKB = S // P  # key blocks per batch (16)
NCORES = 8
GROUPS = [[0, 1], [2, 3], [4, 5], [6, 7]]  # batch-pair replica groups
MASKV = -960.0  # additive pre-scale mask; -30 after the 1/sqrt(D) scale

_compiled_nc = None
_runner = None  # cached (sharded_jit, in_names, out_names, out_avals, n_params)
last_result = None  # kept for compatibility with older test harnesses


def _trace_kernel(tc, out, xT, xn, xqT, mqk, wvT, maskadd):
    nc = tc.nc
    f32 = mybir.dt.float32
    bf16 = mybir.dt.bfloat16
    ts = bass.ts

    with (
        tc.tile_pool(name="sb", bufs=1) as sb,
        tc.tile_pool(name="dram", bufs=1, space="DRAM") as dram,
    ):
        # ---- persistent SBUF ----
        xT_s = sb.tile([P, EC, S], bf16)    # x^T full (scores moving operand
                                            # and V-projection stationary)
        xqT_s = sb.tile([P, EC, D], bf16)   # own-query columns of x^T
        XMT_s = sb.tile([P, EC, D], bf16)   # (x M)^T for own queries, where
                                            # M = Wq^T Wk is folded on host
        xnat_s = sb.tile([P, NKB, D], bf16)  # x natural (k on partitions),
                                             # the moving operand of Z=attn*x
        QT_s = XMT_s  # scores read this exactly like the old Q^T
        mask_s = sb.tile([P, 2 * P], f32)   # additive mask, last 2 key tiles
        mq_s = sb.tile([P, EC, D], bf16)    # M, laid out like a weight
        wv_s = sb.tile([P, EC, D], bf16)

        # ---- input loads (sync queues are FIFO; consumption order) ----
        # mq + xqT interleaved half-granular so the ec-outer XM matmuls
        # stream right behind the DMA during the load-bound first ~12us.
        for ec in range(EC):
            nc.sync.dma_start(mq_s[:, ec, :512], mqk[ts(ec, P), :512])
            nc.sync.dma_start(xqT_s[:, ec, :512], xqT[ts(ec, P), :512])
        for ec in range(EC):
            nc.sync.dma_start(mq_s[:, ec, 512:], mqk[ts(ec, P), 512:])
        for ec in range(EC):
            nc.sync.dma_start(xqT_s[:, ec, 512:], xqT[ts(ec, P), 512:])
        nc.sync.dma_start(mask_s, maskadd)
        for ec in range(EC):
            nc.sync.dma_start(wv_s[:, ec], wvT[ts(ec, P), :])
        for ec in range(EC):
            nc.sync.dma_start(xT_s[:, ec, :H], xT[ts(ec, P), :H])
        for ec in range(EC):
            nc.sync.dma_start(xT_s[:, ec, H:], xT[ts(ec, P), H:])
        # x natural rides the scalar engine's queue, in parallel with the
        # 10MB of sync loads (attention's Z matmuls need it from ~45us)
        for kt in range(NKB):
            nc.scalar.dma_start(xnat_s[:, kt], xn[ts(kt, P), :])

        # One PSUM pool for the whole kernel ("s" ring 4 banks + "big" ring
        # 4 banks): closing a scoped pool mid-kernel acts as a coarse
        # barrier on every pending accumulator copy (~10us PE stall).
        with tc.tile_pool(name="ps", bufs=2, space="PSUM") as ps:
            # ---- XM projection: XMT[e', q] = sum_e M[e, e'] * xqT[e, q] ----
            # (replaces both the old Q^T projection AND the K^T projection +
            # AllGather: scores = Q K^T = x (Wq^T Wk) x^T, so with M folded
            # on the host the device needs no K at all -- the scores contract
            # XMT against the raw x^T.) ec-outer over 4 concurrent PSUM
            # groups so the PE streams behind the initial DMA; chunks
            # alternate "s"/"big" rings so adjacent chunks never share ring
            # slots and boundaries never stall on pending copies.
            def xm_chunk(sc, dh, use_big=False):
                if use_big:
                    bigs = [ps.tile([P, D], f32, tag="big",
                                    name=f"xB{sc}_{dh}_{i}") for i in range(2)]
                    accs = [bigs[0][:, :512], bigs[0][:, 512:],
                            bigs[1][:, :512], bigs[1][:, 512:]]
                else:
                    accs = [ps.tile([P, 512], f32, tag="s", bufs=4,
                                    name=f"xb{sc}_{dh}_{i}") for i in range(4)]
                for ec in range(EC):
                    for i in range(4):
                        nc.tensor.matmul(
                            accs[i], mq_s[:, ec, ts(4 * dh + i, P)],
                            xqT_s[:, ec, ts(sc, 512)],
                            start=(ec == 0), stop=(ec == EC - 1))
                for i in range(4):
                    nc.scalar.copy(XMT_s[:, 4 * dh + i, ts(sc, 512)], accs[i])

            xm_chunk(0, 0)
            xm_chunk(0, 1, use_big=True)
            xm_chunk(1, 0)
            xm_chunk(1, 1, use_big=True)

            # ---- attention, one 128-row query block at a time ----
            # Software-pipelined: S/exp of the NEXT block is traced between the
            # S/exp and transpose/AV of the current one, so the PE has matmul
            # work while ACT/DVE chew through exp and P^T copies.
            inv_sqrt_d = 1.0 / float(np.sqrt(D))

            def s_phase(j):
                nkt = 2 * j + 2          # key tiles (uniform across cores)
                ncols = nkt * P
                nch = (ncols + 511) // 512
                p_sb = sb.tile([P, S], bf16, tag="p_sb", bufs=3)
                pT_sb = sb.tile([P, NKB, P], bf16, tag="pT_sb", bufs=3)
                dsl = sb.tile([P, 4], f32, tag="dsl", bufs=3)
                for ch in range(nch):
                    c0 = ch * 512
                    cw = min(512, ncols - c0)
                    sfull = ps.tile([P, 512], f32, tag="s", bufs=4)
                    sps = sfull[:, :cw]
                    for dc in range(EC):
                        nc.tensor.matmul(
                            sps, QT_s[:, dc, ts(j, P)], xT_s[:, dc, c0:c0 + cw],
                            start=(dc == 0), stop=(dc == EC - 1))
                    if c0 + cw == ncols:  # last chunk holds the 2 maskable tiles
                        nc.vector.tensor_add(
                            sps[:, cw - 2 * P:cw], sps[:, cw - 2 * P:cw], mask_s)
                    nc.scalar.activation(
                        p_sb[:, c0:c0 + cw], sps,
                        mybir.ActivationFunctionType.Exp,
                        scale=inv_sqrt_d,
                        accum_out=dsl[:, ch:ch + 1])
                    # xbar-transpose the finished chunk off the hot engines:
                    # pT_sb[p, kt, q] = p_sb[q, 128*kt + p]
                    nc.sync.dma_start(pT_sb[:, ch * 4:ch * 4 + cw // P, :],
                                      p_sb[:, c0:c0 + cw], transpose=True)
                return p_sb, pT_sb, dsl, nkt, nch

            def av_phase(j, p_sb, pT_sb, dsl, nkt, nch, nsplit=2):
                denom = sb.tile([P, 1], f32, tag="den", bufs=2)
                nc.vector.reduce_sum(denom, dsl[:, :nch], axis=mybir.AxisListType.X)
                recip = sb.tile([P, 1], f32, tag="rcp", bufs=2)
                nc.vector.reciprocal(recip, denom)

                acc = ps.tile([P, D], f32, tag="big")
                for kt in range(nkt):
                    for nh in range(2):
                        nc.tensor.matmul(
                            acc[:, ts(nh, 512)], pT_sb[:, kt, :],
                            V_s[:, kt, ts(nh, 512)],
                            start=(kt == 0), stop=(kt == nkt - 1))
                o_sb = sb.tile([P, D], f32, tag="o_sb", bufs=2)
                # normalize on DVE (idle now), split so the out DMA overlaps
                w = D // nsplit
                for i in range(nsplit):
                    nc.vector.tensor_scalar_mul(
                        o_sb[:, i * w:(i + 1) * w], acc[:, i * w:(i + 1) * w],
                        recip)
                    nc.sync.dma_start(out[j, :, i * w:(i + 1) * w],
                                      o_sb[:, i * w:(i + 1) * w])

            # ---- z/o phases: out = attn (x Wv^T) = ((attn x) Wv^T) ----
            # The Wv projection happens AFTER the attention reduction, on the
            # core's 1024 query rows instead of 2048 duplicated key rows
            # (-27.6us of PE). The row-normalization commutes through both
            # linear maps, so it is folded into the Z->bf16 cast.

            def z_phase(j, p_sb, pT_sb, dsl, nkt, nch):
                denom = sb.tile([P, 1], f32, tag="den", bufs=2)
                nc.vector.reduce_sum(denom, dsl[:, :nch], axis=mybir.AxisListType.X)
                recip = sb.tile([P, 1], f32, tag="rcp", bufs=2)
                nc.vector.reciprocal(recip, denom)

                zacc = ps.tile([P, D], f32, tag="big")
                for kt in range(nkt):
                    for nh in range(2):
                        nc.tensor.matmul(
                            zacc[:, ts(nh, 512)], pT_sb[:, kt, :],
                            xnat_s[:, kt, ts(nh, 512)],
                            start=(kt == 0), stop=(kt == nkt - 1))
                zn = sb.tile([P, D], bf16, tag="zn", bufs=2)
                nc.vector.tensor_scalar_mul(zn[:, :512], zacc[:, :512], recip)
                nc.vector.tensor_scalar_mul(zn[:, 512:], zacc[:, 512:], recip)
                znT = sb.tile([P, EC, P], bf16, tag="znT", bufs=2)
                nc.sync.dma_start(znT[:, 0:4, :], zn[:, :512], transpose=True)
                nc.sync.dma_start(znT[:, 4:8, :], zn[:, 512:], transpose=True)
                return znT

            def o_phase(j, znT, nsplit=2):
                oacc = ps.tile([P, D], f32, tag="big")
                for ec in range(EC):
                    for nh in range(2):
                        nc.tensor.matmul(
                            oacc[:, ts(nh, 512)], znT[:, ec, :],
                            wv_s[:, ec, ts(nh, 512)],
                            start=(ec == 0), stop=(ec == EC - 1))
                o_sb = sb.tile([P, D], f32, tag="o_sb", bufs=2)
                # copy + store both ride the scalar engine (sync is ~70%
                # busy with loads and the two transpose families)
                w = D // nsplit
                for i in range(nsplit):
                    nc.scalar.copy(o_sb[:, i * w:(i + 1) * w],
                                   oacc[:, i * w:(i + 1) * w])
                    nc.scalar.dma_start(out[j, :, i * w:(i + 1) * w],
                                        o_sb[:, i * w:(i + 1) * w])

            # software pipeline: s(next2), z(next), o(cur) rotate so every PE
            # stage has the prior block's exp/transpose latencies covered
            sstates, zstates = {}, {}

            def run_s(j):
                sstates[j] = s_phase(j)

            def run_z(j):
                zstates[j] = z_phase(j, *sstates.pop(j))

            def run_o(j, nsplit=2):
                o_phase(j, zstates.pop(j), nsplit=nsplit)

            run_s(7); run_s(6); run_z(7); run_s(5); run_z(6); run_o(7)
            run_s(0); run_z(5); run_o(6); run_s(1); run_z(0); run_o(5)
            run_s(2); run_z(1); run_o(0); run_s(3); run_z(2); run_o(1)
            run_s(4); run_z(3); run_z(4); run_o(2); run_o(3)
            run_o(4, nsplit=4)

def build_nc(debug=False):
    nc = bacc.Bacc("TRN2", target_bir_lowering=False, debug=debug,
                   enable_asserts=False, num_devices=NCORES)
    bf16 = mybir.dt.bfloat16
    f32 = mybir.dt.float32
    xT = nc.dram_tensor("xT", (D, S), bf16, kind="ExternalInput").ap()
    xn = nc.dram_tensor("xn", (S, D), bf16, kind="ExternalInput").ap()
    xqT = nc.dram_tensor("xqT", (D, D), bf16, kind="ExternalInput").ap()
    mqk = nc.dram_tensor("mqk", (D, D), bf16, kind="ExternalInput").ap()
    wvT = nc.dram_tensor("wvT", (D, D), bf16, kind="ExternalInput").ap()
    maskadd = nc.dram_tensor("maskadd", (P, 2 * P), f32,
                             kind="ExternalInput").ap()
    out = nc.dram_tensor("out", (NQB, P, D), f32, kind="ExternalOutput").ap()
    with tile.TileContext(nc) as tc:
        _trace_kernel(tc, out, xT, xn, xqT, mqk, wvT, maskadd)
    nc.compile()
    return nc


def _get_compiled():
    global _compiled_nc
    if _compiled_nc is None:
        _compiled_nc = build_nc(debug=False)
    return _compiled_nc


def _get_runner():
    """Jit-once shard_map runner over the 8 NeuronCores.

    Mirrors bass2jax.run_bass_via_pjrt's multi-core branch, but caches the
    jitted executable so repeat kernel() calls skip retracing/recompiling.
    """
    global _runner
    if _runner is not None:
        return _runner
    import jax
    from jax.experimental.shard_map import shard_map
    from jax.sharding import Mesh, PartitionSpec
    from concourse import bass2jax

    nc = _get_compiled()
    bass2jax.install_neuronx_cc_hook()

    partition_name = (nc.partition_id_tensor.name
                      if nc.partition_id_tensor else None)
    in_names, out_names, out_avals, zero_outs = [], [], [], []
    for alloc in nc.m.functions[0].allocations:
        if not isinstance(alloc, mybir.MemoryLocationSet):
            continue
        name = alloc.memorylocations[0].name
        if alloc.kind == "ExternalInput":
            if name != partition_name:
                in_names.append(name)
        elif alloc.kind == "ExternalOutput":
            shape = tuple(alloc.tensor_shape)
            dtype = mybir.dt.np(alloc.dtype)
            out_names.append(name)
            out_avals.append(jax.core.ShapedArray(shape, dtype))
            zero_outs.append(np.zeros(shape, dtype))
    n_params = len(in_names)
    all_in_names = list(in_names) + list(out_names)
    if partition_name is not None:
        all_in_names.append(partition_name)
    donate = tuple(range(n_params, n_params + len(out_names)))

    def _body(*args):
        operands = list(args)
        if partition_name is not None:
            operands.append(bass2jax.partition_id_tensor())
        outs = bass2jax._bass_exec_p.bind(
            *operands,
            out_avals=tuple(out_avals),
            in_names=tuple(all_in_names),
            out_names=tuple(out_names),
            lowering_input_output_aliases=(),
            sim_require_finite=True,
            sim_require_nnan=True,
            nc=nc,
        )
        return tuple(outs)

    devices = jax.devices()[:NCORES]
    mesh = Mesh(np.asarray(devices), ("core",))
    nin = n_params + len(out_names)
    sharded = jax.jit(
        shard_map(_body, mesh=mesh,
                  in_specs=(PartitionSpec("core"),) * nin,
                  out_specs=(PartitionSpec("core"),) * len(out_names),
                  check_rep=False),
        donate_argnums=donate, keep_unused=True)
    _runner = (sharded, in_names, out_names, out_avals, n_params, zero_outs, mesh)
    return _runner


def run_device(in_maps):
    """Execute the compiled NEFF on all 8 cores; returns per-core output dicts."""
    sharded, in_names, out_names, out_avals, n_params, zero_outs, _ = _get_runner()
    concat_in = [
        np.concatenate([np.asarray(in_maps[c][nm]) for c in range(NCORES)], axis=0)
        for nm in in_names
    ]
    concat_zeros = [
        np.zeros((NCORES * z.shape[0], *z.shape[1:]), z.dtype) for z in zero_outs
    ]
    out_arrs = sharded(*concat_in, *concat_zeros)
    return [
        {nm: np.asarray(out_arrs[i]).reshape(NCORES, *out_avals[i].shape)[c]
         for i, nm in enumerate(out_names)}
        for c in range(NCORES)
    ]


def make_in_maps(x):
    """Per-core host-side slicing + layout prep (no matmul math here)."""
    x = np.asarray(x, dtype=np.float32)
    r = np.arange(P)
    tri_add = np.where(r[None, :] <= r[:, None], 0.0, MASKV).astype(np.float32)
    mask_h = []
    for h in range(2):
        if h == 0:
            blk = np.concatenate(
                [tri_add, np.full((P, P), MASKV, np.float32)], axis=1)
        else:
            blk = np.concatenate([np.zeros((P, P), np.float32), tri_add], axis=1)
        mask_h.append(np.ascontiguousarray(blk).astype(np.float32))

    in_maps = []
    xT_b = {}
    for c in range(NCORES):
        b, h = c // 2, c % 2
        if b not in xT_b:
            xT_b[b] = np.ascontiguousarray(x[b].T).astype(BF16)
        blocks = [2 * j + h for j in range(NQB)]
        xq = np.concatenate([x[b][g * P:(g + 1) * P] for g in blocks], axis=0)
        xqT = np.ascontiguousarray(xq.T).astype(BF16)
        in_maps.append({
            "xT": xT_b[b],
            "xn": np.ascontiguousarray(x[b]).astype(BF16),
            "xqT": xqT,
            "maskadd": mask_h[h],
        })
    return in_maps


def make_weight_map(inputs):
    """Pre-transposed bf16 weights keyed by NEFF input name."""
    Wq = np.asarray(inputs["Wq"], np.float32)
    Wk = np.asarray(inputs["Wk"], np.float32)
    return {
        "mqk": np.ascontiguousarray(Wq.T @ Wk).astype(BF16),
        "wvT": np.ascontiguousarray(np.asarray(inputs["Wv"], np.float32).T).astype(BF16),
    }


def kernel(x, Wq, bq, Wk, bk, Wv, bv, mask):
    global last_result
    x = np.asarray(x, np.float32)
    Wq = np.asarray(Wq, np.float32)
    Wk = np.asarray(Wk, np.float32)
    Wv = np.asarray(Wv, np.float32)
    bq = np.asarray(bq, np.float32)
    bk = np.asarray(bk, np.float32)
    bv = np.asarray(bv, np.float32)
    mask = np.asarray(mask)

    causal = bool(np.array_equal(mask != 0, np.tril(np.ones(mask.shape, bool))))
    if np.any(bq) or np.any(bk) or not causal:
        return _np_reference(x, Wq, bq, Wk, bk, Wv, bv, mask)

    in_maps = make_in_maps(x)
    wT = {
        "mqk": np.ascontiguousarray(Wq.T @ Wk).astype(BF16),
        "wvT": np.ascontiguousarray(Wv.T).astype(BF16),
    }
    for m in in_maps:
        m.update(wT)

    results = None
    for attempt in range(3):  # remote NeuronCores occasionally wedge transiently
        try:
            results = run_device(in_maps)
            if any(not np.isfinite(np.asarray(r["out"])).all() for r in results):
                raise FloatingPointError("transient non-finite device output")
            break
        except Exception:
            if attempt == 2:
                raise
            time.sleep(30)

    out = np.empty((B * S, D), np.float32)
    for c in range(NCORES):
        b, h = c // 2, c % 2
        o = np.asarray(results[c]["out"], np.float32)
        for j in range(NQB):
            g = 2 * j + h
            out[b * S + g * P: b * S + (g + 1) * P] = o[j]
    if np.any(bv):
        out = out + bv[None, :]  # attn rows sum to 1, so bv adds exactly
    return out


def _np_reference(x, Wq, bq, Wk, bk, Wv, bv, mask):
    outs = []
    for b in range(x.shape[0]):
        xb = x[b]
        Q = xb @ Wq.T + bq
        K = xb @ Wk.T + bk
        V = xb @ Wv.T + bv
        Sc = (Q @ K.T) / np.float32(np.sqrt(x.shape[2]))
        Sc = np.where(mask == 0, np.float32(-1e9), Sc)
        Sc = Sc - Sc.max(axis=1, keepdims=True)
        E = np.exp(Sc)
        A = E / E.sum(axis=1, keepdims=True)
        outs.append(A @ V)
    return np.concatenate(outs, axis=0).astype(np.float32)


# revision 40
# speedup vs baseline: 1.0678x; 1.0678x over previous
"""Causal self-attention (B=4, S=2048, D=1024, single 1024-wide head) on 8 TRN2 cores.

Sharding: core c -> batch b=c//2, parity h=c%2; each core computes the 8
query blocks {h, h+2, ..., h+14} (128 rows each) of its batch. Key-extents
padded to 256*(j+1) keep the program identical on every core; causality
lives in per-core additive-mask input data, not control flow.

The key algebraic move: with zero q/k biases, scores = Q K^T =
x (Wq^T Wk) x^T, so M = Wq^T Wk is folded ON THE HOST (weights-only) and
the device needs no K projection and no Q projection -- just one
XM = x M projection of the core's own queries (ec-outer over 4 PSUM groups,
paced behind the initial DMA), then scores contract XM^T against the raw
x^T. V is projected in full per core. Everything is local: no collectives
(a pair-wise AllGather costs ~45us on the serialized CC ring, which is why
sharded K/V projections lose), no readbacks, ~10MB of input DMA.

Phases: XM projection -> V projection (the two biggest s_phases hoisted
into its tail so their exp/transpose latencies hide under V matmuls) ->
attention, software-pipelined with the tail run 2 s_phases ahead so no
av_phase waits on an exp->transpose chain. Loads are issued in exact
consumption order at half-tile granularity (DMA queues are FIFO).

All matmuls run on the PE in bf16 with fp32 PSUM accumulation. Softmax
skips max-subtraction (scores are ~N(0,1); exp stays in fp32 range) so the
denominator comes free from the Exp activation's accumulate output.
"""

import time

import numpy as np
import ml_dtypes

import concourse.bass as bass
import concourse.bacc as bacc
import concourse.tile as tile
from concourse import mybir
from concourse import bass_utils

BF16 = ml_dtypes.bfloat16
P = 128
B, S, D = 4, 2048, 1024
H = S // 2   # keys owned per core (half a batch)
EC = D // P  # contraction chunks (8)
NQB = 8      # query blocks per core
NKB = S // P  # key blocks per batch (16)
NCORES = 8
GROUPS = [[0, 1], [2, 3], [4, 5], [6, 7]]  # batch-pair replica groups
MASKV = -960.0  # additive pre-scale mask; -30 after the 1/sqrt(D) scale

_compiled_nc = None
_runner = None  # cached (sharded_jit, in_names, out_names, out_avals, n_params)
last_result = None  # kept for compatibility with older test harnesses


def _trace_kernel(tc, out, xT, xn, xqT, mqk, wvT, maskadd):
    nc = tc.nc
    f32 = mybir.dt.float32
    bf16 = mybir.dt.bfloat16
    ts = bass.ts

    with (
        tc.tile_pool(name="sb", bufs=1) as sb,
        tc.tile_pool(name="dram", bufs=1, space="DRAM") as dram,
    ):
        # ---- persistent SBUF ----
        xT_s = sb.tile([P, EC, S], bf16)    # x^T full (scores moving operand
                                            # and V-projection stationary)
        xqT_s = sb.tile([P, EC, D], bf16)   # own-query columns of x^T
        XMT_s = sb.tile([P, EC, D], bf16)   # (x M)^T for own queries, where
                                            # M = Wq^T Wk is folded on host
        xnat_s = sb.tile([P, NKB, D], bf16)  # x natural (k on partitions),
                                             # the moving operand of Z=attn*x
        QT_s = XMT_s  # scores read this exactly like the old Q^T
        mask_s = sb.tile([P, 2 * P], f32)   # additive mask, last 2 key tiles
        mq_s = sb.tile([P, EC, D], bf16)    # M, laid out like a weight
        wv_s = sb.tile([P, EC, D], bf16)

        # ---- input loads (sync queues are FIFO; consumption order) ----
        # mq + xqT interleaved half-granular so the ec-outer XM matmuls
        # stream right behind the DMA during the load-bound first ~12us.
        for ec in range(EC):
            nc.sync.dma_start(mq_s[:, ec, :512], mqk[ts(ec, P), :512])
            nc.sync.dma_start(xqT_s[:, ec, :512], xqT[ts(ec, P), :512])
        for ec in range(EC):
            nc.sync.dma_start(mq_s[:, ec, 512:], mqk[ts(ec, P), 512:])
        for ec in range(EC):
            nc.sync.dma_start(xqT_s[:, ec, 512:], xqT[ts(ec, P), 512:])
        nc.sync.dma_start(mask_s, maskadd)
        for ec in range(EC):
            nc.sync.dma_start(wv_s[:, ec], wvT[ts(ec, P), :])
        for ec in range(EC):
            nc.sync.dma_start(xT_s[:, ec, :H], xT[ts(ec, P), :H])
        for ec in range(EC):
            nc.sync.dma_start(xT_s[:, ec, H:], xT[ts(ec, P), H:])
        # x natural rides the scalar engine's queue, in parallel with the
        # 10MB of sync loads (attention's Z matmuls need it from ~45us)
        for kt in range(NKB):
            nc.scalar.dma_start(xnat_s[:, kt], xn[ts(kt, P), :])

        # One PSUM pool for the whole kernel ("s" ring 4 banks + "big" ring
        # 4 banks): closing a scoped pool mid-kernel acts as a coarse
        # barrier on every pending accumulator copy (~10us PE stall).
        with tc.tile_pool(name="ps", bufs=2, space="PSUM") as ps:
            # ---- XM projection: XMT[e', q] = sum_e M[e, e'] * xqT[e, q] ----
            # (replaces both the old Q^T projection AND the K^T projection +
            # AllGather: scores = Q K^T = x (Wq^T Wk) x^T, so with M folded
            # on the host the device needs no K at all -- the scores contract
            # XMT against the raw x^T.) ec-outer over 4 concurrent PSUM
            # groups so the PE streams behind the initial DMA; chunks
            # alternate "s"/"big" rings so adjacent chunks never share ring
            # slots and boundaries never stall on pending copies.
            def xm_chunk(sc, dh, use_big=False):
                if use_big:
                    bigs = [ps.tile([P, D], f32, tag="big",
                                    name=f"xB{sc}_{dh}_{i}") for i in range(2)]
                    accs = [bigs[0][:, :512], bigs[0][:, 512:],
                            bigs[1][:, :512], bigs[1][:, 512:]]
                else:
                    accs = [ps.tile([P, 512], f32, tag="s", bufs=4,
                                    name=f"xb{sc}_{dh}_{i}") for i in range(4)]
                for ec in range(EC):
                    for i in range(4):
                        nc.tensor.matmul(
                            accs[i], mq_s[:, ec, ts(4 * dh + i, P)],
                            xqT_s[:, ec, ts(sc, 512)],
                            start=(ec == 0), stop=(ec == EC - 1))
                for i in range(4):
                    nc.scalar.copy(XMT_s[:, 4 * dh + i, ts(sc, 512)], accs[i])

            xm_chunk(0, 0)
            xm_chunk(0, 1, use_big=True)
            xm_chunk(1, 0)
            xm_chunk(1, 1, use_big=True)

            # ---- attention, one 128-row query block at a time ----
            # Software-pipelined: S/exp of the NEXT block is traced between the
            # S/exp and transpose/AV of the current one, so the PE has matmul
            # work while ACT/DVE chew through exp and P^T copies.
            inv_sqrt_d = 1.0 / float(np.sqrt(D))

            def s_phase(j):
                nkt = 2 * j + 2          # key tiles (uniform across cores)
                ncols = nkt * P
                nch = (ncols + 511) // 512
                p_sb = sb.tile([P, S], bf16, tag="p_sb", bufs=3)
                pT_sb = sb.tile([P, NKB, P], bf16, tag="pT_sb", bufs=3)
                dsl = sb.tile([P, 4], f32, tag="dsl", bufs=3)
                for ch in range(nch):
                    c0 = ch * 512
                    cw = min(512, ncols - c0)
                    sfull = ps.tile([P, 512], f32, tag="s", bufs=4)
                    sps = sfull[:, :cw]
                    for dc in range(EC):
                        nc.tensor.matmul(
                            sps, QT_s[:, dc, ts(j, P)], xT_s[:, dc, c0:c0 + cw],
                            start=(dc == 0), stop=(dc == EC - 1))
                    if c0 + cw == ncols:  # last chunk holds the 2 maskable tiles
                        nc.vector.tensor_add(
                            sps[:, cw - 2 * P:cw], sps[:, cw - 2 * P:cw], mask_s)
                    nc.scalar.activation(
                        p_sb[:, c0:c0 + cw], sps,
                        mybir.ActivationFunctionType.Exp,
                        scale=inv_sqrt_d,
                        accum_out=dsl[:, ch:ch + 1])
                    # xbar-transpose the finished chunk off the hot engines:
                    # pT_sb[p, kt, q] = p_sb[q, 128*kt + p]
                    nc.sync.dma_start(pT_sb[:, ch * 4:ch * 4 + cw // P, :],
                                      p_sb[:, c0:c0 + cw], transpose=True)
                return p_sb, pT_sb, dsl, nkt, nch

            def av_phase(j, p_sb, pT_sb, dsl, nkt, nch, nsplit=2):
                denom = sb.tile([P, 1], f32, tag="den", bufs=2)
                nc.vector.reduce_sum(denom, dsl[:, :nch], axis=mybir.AxisListType.X)
                recip = sb.tile([P, 1], f32, tag="rcp", bufs=2)
                nc.vector.reciprocal(recip, denom)

                acc = ps.tile([P, D], f32, tag="big")
                for kt in range(nkt):
                    for nh in range(2):
                        nc.tensor.matmul(
                            acc[:, ts(nh, 512)], pT_sb[:, kt, :],
                            V_s[:, kt, ts(nh, 512)],
                            start=(kt == 0), stop=(kt == nkt - 1))
                o_sb = sb.tile([P, D], f32, tag="o_sb", bufs=2)
                # normalize on DVE (idle now), split so the out DMA overlaps
                w = D // nsplit
                for i in range(nsplit):
                    nc.vector.tensor_scalar_mul(
                        o_sb[:, i * w:(i + 1) * w], acc[:, i * w:(i + 1) * w],
                        recip)
                    nc.sync.dma_start(out[j, :, i * w:(i + 1) * w],
                                      o_sb[:, i * w:(i + 1) * w])

            # ---- z/o phases: out = attn (x Wv^T) = ((attn x) Wv^T) ----
            # The Wv projection happens AFTER the attention reduction, on the
            # core's 1024 query rows instead of 2048 duplicated key rows
            # (-27.6us of PE). The row-normalization commutes through both
            # linear maps, so it is folded into the Z->bf16 cast.

            def z_phase(j, p_sb, pT_sb, dsl, nkt, nch):
                denom = sb.tile([P, 1], f32, tag="den", bufs=2)
                nc.vector.reduce_sum(denom, dsl[:, :nch], axis=mybir.AxisListType.X)
                recip = sb.tile([P, 1], f32, tag="rcp", bufs=2)
                nc.vector.reciprocal(recip, denom)

                zacc = ps.tile([P, D], f32, tag="big")
                for kt in range(nkt):
                    for nh in range(2):
                        nc.tensor.matmul(
                            zacc[:, ts(nh, 512)], pT_sb[:, kt, :],
                            xnat_s[:, kt, ts(nh, 512)],
                            start=(kt == 0), stop=(kt == nkt - 1))
                zn = sb.tile([P, D], bf16, tag="zn", bufs=2)
                nc.vector.tensor_scalar_mul(zn[:, :512], zacc[:, :512], recip)
                nc.vector.tensor_scalar_mul(zn[:, 512:], zacc[:, 512:], recip)
                znT = sb.tile([P, EC, P], bf16, tag="znT", bufs=2)
                nc.sync.dma_start(znT[:, 0:4, :], zn[:, :512], transpose=True)
                nc.sync.dma_start(znT[:, 4:8, :], zn[:, 512:], transpose=True)
                return znT

            def o_phase(j, znT):
                oacc = ps.tile([P, D], f32, tag="big")
                for ec in range(EC):
                    for nh in range(2):
                        nc.tensor.matmul(
                            oacc[:, ts(nh, 512)], znT[:, ec, :],
                            wv_s[:, ec, ts(nh, 512)],
                            start=(ec == 0), stop=(ec == EC - 1))
                o_sb = sb.tile([P, D], f32, tag="o_sb", bufs=2)
                nc.scalar.copy(o_sb[:, :512], oacc[:, :512])
                nc.sync.dma_start(out[j, :, :512], o_sb[:, :512])
                nc.scalar.copy(o_sb[:, 512:], oacc[:, 512:])
                nc.sync.dma_start(out[j, :, 512:], o_sb[:, 512:])

            # software pipeline: s(next2), z(next), o(cur) rotate so every PE
            # stage has the prior block's exp/transpose latencies covered
            sstates, zstates = {}, {}

            def run_s(j):
                sstates[j] = s_phase(j)

            def run_z(j):
                zstates[j] = z_phase(j, *sstates.pop(j))

            def run_o(j):
                o_phase(j, zstates.pop(j))

            run_s(7); run_s(6); run_z(7); run_s(5); run_z(6); run_o(7)
            run_s(0); run_z(5); run_o(6); run_s(1); run_z(0); run_o(5)
            run_s(2); run_z(1); run_o(0); run_s(3); run_z(2); run_o(1)
            run_s(4); run_z(3); run_o(2); run_z(4); run_o(3); run_o(4)

def build_nc(debug=False):
    nc = bacc.Bacc("TRN2", target_bir_lowering=False, debug=debug,
                   enable_asserts=False, num_devices=NCORES)
    bf16 = mybir.dt.bfloat16
    f32 = mybir.dt.float32
    xT = nc.dram_tensor("xT", (D, S), bf16, kind="ExternalInput").ap()
    xn = nc.dram_tensor("xn", (S, D), bf16, kind="ExternalInput").ap()
    xqT = nc.dram_tensor("xqT", (D, D), bf16, kind="ExternalInput").ap()
    mqk = nc.dram_tensor("mqk", (D, D), bf16, kind="ExternalInput").ap()
    wvT = nc.dram_tensor("wvT", (D, D), bf16, kind="ExternalInput").ap()
    maskadd = nc.dram_tensor("maskadd", (P, 2 * P), f32,
                             kind="ExternalInput").ap()
    out = nc.dram_tensor("out", (NQB, P, D), f32, kind="ExternalOutput").ap()
    with tile.TileContext(nc) as tc:
        _trace_kernel(tc, out, xT, xn, xqT, mqk, wvT, maskadd)
    nc.compile()
    return nc


def _get_compiled():
    global _compiled_nc
    if _compiled_nc is None:
        _compiled_nc = build_nc(debug=False)
    return _compiled_nc


def _get_runner():
    """Jit-once shard_map runner over the 8 NeuronCores.

    Mirrors bass2jax.run_bass_via_pjrt's multi-core branch, but caches the
    jitted executable so repeat kernel() calls skip retracing/recompiling.
    """
    global _runner
    if _runner is not None:
        return _runner
    import jax
    from jax.experimental.shard_map import shard_map
    from jax.sharding import Mesh, PartitionSpec
    from concourse import bass2jax

    nc = _get_compiled()
    bass2jax.install_neuronx_cc_hook()

    partition_name = (nc.partition_id_tensor.name
                      if nc.partition_id_tensor else None)
    in_names, out_names, out_avals, zero_outs = [], [], [], []
    for alloc in nc.m.functions[0].allocations:
        if not isinstance(alloc, mybir.MemoryLocationSet):
            continue
        name = alloc.memorylocations[0].name
        if alloc.kind == "ExternalInput":
            if name != partition_name:
                in_names.append(name)
        elif alloc.kind == "ExternalOutput":
            shape = tuple(alloc.tensor_shape)
            dtype = mybir.dt.np(alloc.dtype)
            out_names.append(name)
            out_avals.append(jax.core.ShapedArray(shape, dtype))
            zero_outs.append(np.zeros(shape, dtype))
    n_params = len(in_names)
    all_in_names = list(in_names) + list(out_names)
    if partition_name is not None:
        all_in_names.append(partition_name)
    donate = tuple(range(n_params, n_params + len(out_names)))

    def _body(*args):
        operands = list(args)
        if partition_name is not None:
            operands.append(bass2jax.partition_id_tensor())
        outs = bass2jax._bass_exec_p.bind(
            *operands,
            out_avals=tuple(out_avals),
            in_names=tuple(all_in_names),
            out_names=tuple(out_names),
            lowering_input_output_aliases=(),
            sim_require_finite=True,
            sim_require_nnan=True,
            nc=nc,
        )
        return tuple(outs)

    devices = jax.devices()[:NCORES]
    mesh = Mesh(np.asarray(devices), ("core",))
    nin = n_params + len(out_names)
    sharded = jax.jit(
        shard_map(_body, mesh=mesh,
                  in_specs=(PartitionSpec("core"),) * nin,
                  out_specs=(PartitionSpec("core"),) * len(out_names),
                  check_rep=False),
        donate_argnums=donate, keep_unused=True)
    _runner = (sharded, in_names, out_names, out_avals, n_params, zero_outs, mesh)
    return _runner


def run_device(in_maps):
    """Execute the compiled NEFF on all 8 cores; returns per-core output dicts."""
    sharded, in_names, out_names, out_avals, n_params, zero_outs, _ = _get_runner()
    concat_in = [
        np.concatenate([np.asarray(in_maps[c][nm]) for c in range(NCORES)], axis=0)
        for nm in in_names
    ]
    concat_zeros = [
        np.zeros((NCORES * z.shape[0], *z.shape[1:]), z.dtype) for z in zero_outs
    ]
    out_arrs = sharded(*concat_in, *concat_zeros)
    return [
        {nm: np.asarray(out_arrs[i]).reshape(NCORES, *out_avals[i].shape)[c]
         for i, nm in enumerate(out_names)}
        for c in range(NCORES)
    ]


def make_in_maps(x):
    """Per-core host-side slicing + layout prep (no matmul math here)."""
    x = np.asarray(x, dtype=np.float32)
    r = np.arange(P)
    tri_add = np.where(r[None, :] <= r[:, None], 0.0, MASKV).astype(np.float32)
    mask_h = []
    for h in range(2):
        if h == 0:
            blk = np.concatenate(
                [tri_add, np.full((P, P), MASKV, np.float32)], axis=1)
        else:
            blk = np.concatenate([np.zeros((P, P), np.float32), tri_add], axis=1)
        mask_h.append(np.ascontiguousarray(blk).astype(np.float32))

    in_maps = []
    xT_b = {}
    for c in range(NCORES):
        b, h = c // 2, c % 2
        if b not in xT_b:
            xT_b[b] = np.ascontiguousarray(x[b].T).astype(BF16)
        blocks = [2 * j + h for j in range(NQB)]
        xq = np.concatenate([x[b][g * P:(g + 1) * P] for g in blocks], axis=0)
        xqT = np.ascontiguousarray(xq.T).astype(BF16)
        in_maps.append({
            "xT": xT_b[b],
            "xn": np.ascontiguousarray(x[b]).astype(BF16),
            "xqT": xqT,
            "maskadd": mask_h[h],
        })
    return in_maps


def make_weight_map(inputs):
    """Pre-transposed bf16 weights keyed by NEFF input name."""
    Wq = np.asarray(inputs["Wq"], np.float32)
    Wk = np.asarray(inputs["Wk"], np.float32)
    return {
        "mqk": np.ascontiguousarray(Wq.T @ Wk).astype(BF16),
        "wvT": np.ascontiguousarray(np.asarray(inputs["Wv"], np.float32).T).astype(BF16),
    }


def kernel(x, Wq, bq, Wk, bk, Wv, bv, mask):
    global last_result
    x = np.asarray(x, np.float32)
    Wq = np.asarray(Wq, np.float32)
    Wk = np.asarray(Wk, np.float32)
    Wv = np.asarray(Wv, np.float32)
    bq = np.asarray(bq, np.float32)
    bk = np.asarray(bk, np.float32)
    bv = np.asarray(bv, np.float32)
    mask = np.asarray(mask)

    causal = bool(np.array_equal(mask != 0, np.tril(np.ones(mask.shape, bool))))
    if np.any(bq) or np.any(bk) or not causal:
        return _np_reference(x, Wq, bq, Wk, bk, Wv, bv, mask)

    in_maps = make_in_maps(x)
    wT = {
        "mqk": np.ascontiguousarray(Wq.T @ Wk).astype(BF16),
        "wvT": np.ascontiguousarray(Wv.T).astype(BF16),
    }
    for m in in_maps:
        m.update(wT)

    results = None
    for attempt in range(3):  # remote NeuronCores occasionally wedge transiently
        try:
            results = run_device(in_maps)
            if any(not np.isfinite(np.asarray(r["out"])).all() for r in results):
                raise FloatingPointError("transient non-finite device output")
            break
        except Exception:
            if attempt == 2:
                raise
            time.sleep(30)

    out = np.empty((B * S, D), np.float32)
    for c in range(NCORES):
        b, h = c // 2, c % 2
        o = np.asarray(results[c]["out"], np.float32)
        for j in range(NQB):
            g = 2 * j + h
            out[b * S + g * P: b * S + (g + 1) * P] = o[j]
    if np.any(bv):
        out = out + bv[None, :]  # attn rows sum to 1, so bv adds exactly
    return out


def _np_reference(x, Wq, bq, Wk, bk, Wv, bv, mask):
    outs = []
    for b in range(x.shape[0]):
        xb = x[b]
        Q = xb @ Wq.T + bq
        K = xb @ Wk.T + bk
        V = xb @ Wv.T + bv
        Sc = (Q @ K.T) / np.float32(np.sqrt(x.shape[2]))
        Sc = np.where(mask == 0, np.float32(-1e9), Sc)
        Sc = Sc - Sc.max(axis=1, keepdims=True)
        E = np.exp(Sc)
        A = E / E.sum(axis=1, keepdims=True)
        outs.append(A @ V)
    return np.concatenate(outs, axis=0).astype(np.float32)
